# revision 34
# baseline (speedup 1.0000x reference)
"""Trainium2 Bass kernel for masked candidate-span attention (ragged_sequence).

Math (per char n):
  s_v = x_n . M_v  for all v in [0,96), M = pos_embed @ W   (fp16 matmul)
  cnt_v = #{c : idx_c == v and mask_c}   (one-hot equality + add tree)
  w_v = cnt_v * exp(s_v);  Z = sum_v w;  ctx = (w/Z) @ pos_embed
  Rows with no masked-in candidate or l >= seq_len output 0.

Sharding: pure data parallel over batch (2 batches per core x 8 cores).

Layout: char n in [0, 8192) maps to (partition p = n//64, column i = n%64) so
every DMA is 128 partitions x contiguous bytes. A supertile is 4 columns
(512 chars). x is cast f32->fp16 on Pool/Act, transposed by the DMA XBAR
(no PE transposes / PSUM drains on the x path). The one-hot c-tail is summed
by PSUM-accumulating PE transposes; softmax normalization happens in the
scaled PSUM->SBUF output drains.
"""
import os
import sys

import numpy as np

sys.path.insert(0, "/opt/trn_rl_repo")
_HERE = os.path.dirname(os.path.abspath(__file__))
sys.path.insert(0, _HERE)

from contextlib import ExitStack

import concourse.bass as bass  # noqa: E402
import concourse.mybir as mybir  # noqa: E402
from concourse.tile import TileContext  # noqa: E402

# --- walrus workaround: cap sync waits per instruction ---------------------
import concourse.tile as _tile_mod  # noqa: E402
import bass_rust as _br  # noqa: E402
from concourse.vector_clock import ScopedClock  # noqa: E402


def _patched_drain_and_barrier(self, tick_clock, wait_clock):
    nc = self.nc
    probe = mybir.InstNoOp(name=nc.get_next_instruction_name(), ins=[], outs=[])
    probe.engine = mybir.EngineType.SP
    wait_clock.add_sem_waits(probe, ScopedClock({None: tick_clock.global_clock}))
    waits = list(probe.sync_info.on_wait)
    assert self.sems is not None
    by_num = {h.num: h for h in self.sems.allocated().values()}
    for w in waits:
        nc.sync.wait_ge(by_num[w.id], w.wait_value)
    nc.sync.drain()
    nc.all_engine_barrier()
    popped = nc._tile_sem_poison_stack.pop()
    assert popped is self._sem_poison
    nc.clear_and_free_semaphores(list(self.sems.allocated().values()))
    nc.all_engine_barrier()


_tile_mod.TileContext._drain_and_barrier = _patched_drain_and_barrier


def split_excess_waits(nc):
    for f in nc.m.functions:
        for bb in f.blocks:
            out = []
            changed = False
            for inst in bb.instructions:
                si = inst.sync_info
                waits = list(si.on_wait) if si is not None else []
                cap = 2 if isinstance(inst, _br.InstEventSemaphore) else 1
                if len(waits) > cap:
                    excess, keep = waits[:-cap], waits[-cap:]
                    for k in range(0, len(excess), 2):
                        ev = _br.InstEventSemaphore(
                            name=f"{inst.name}-wsplit{k}", ins=[], outs=[])
                        ev.engine = inst.engine
                        ev.sync_info = _br.SyncInfo(on_wait=excess[k:k + 2],
                                                    on_update=[])
                        out.append(ev)
                    inst.sync_info = _br.SyncInfo(on_wait=keep,
                                                  on_update=list(si.on_update))
                    changed = True
                out.append(inst)
            if changed:
                bb.instructions = out


# --- problem constants -----------------------------------------------------
B, L, C = 16, 4096, 9
DI, DO, V = 512, 128, 96
NCORES = 8
BLOC = B // NCORES          # batches per core
NLOC = BLOC * L             # chars per core (8192)
CPP = NLOC // 128           # chars per partition (64)
NSUP = CPP // 8             # 8 super-tiles (8 columns x 128 partitions each)
SW = 8                      # columns per supertile

f32 = mybir.dt.float32
f32r = mybir.dt.float32r
f16 = mybir.dt.float16
bf16 = mybir.dt.bfloat16
i32 = mybir.dt.int32
i16 = mybir.dt.int16
u8 = mybir.dt.uint8
Alu = mybir.AluOpType
Act = mybir.ActivationFunctionType
Ax = mybir.AxisListType

# split of the fp16 cast columns between Pool (GpSimd) and Act (Scalar)
CAST_POOL_COLS = 1280


def build_kernel():
    nc = bass.Bass()
    x_d = nc.declare_dram_parameter("x", [NLOC, DI], f32r, isOutput=False)
    idx_d = nc.declare_dram_parameter("cand_idx", [NLOC, C], i32, isOutput=False)
    msk_d = nc.declare_dram_parameter("cand_mask", [NLOC, C], u8, isOutput=False)
    w_d = nc.declare_dram_parameter("W", [DO, DI], f32, isOutput=False)
    pos_d = nc.declare_dram_parameter("pos_embed", [V, DO], f32, isOutput=False)
    len_d = nc.declare_dram_parameter("word_seq_len", [1, BLOC], i32, isOutput=False)
    out_d = nc.declare_dram_parameter("out", [NLOC, DO], f32, isOutput=True)

    with TileContext(nc) as tc, ExitStack() as es:
        cpool = es.enter_context(tc.tile_pool(name="consts", bufs=1))
        # ---- constants ----
        io_r = cpool.tile([128, 128], i32)
        io_c = cpool.tile([128, 1], i32)
        nc.gpsimd.iota(io_r[:], pattern=[[1, 128]], base=0, channel_multiplier=0)
        nc.gpsimd.iota(io_c[:], pattern=[[0, 1]], base=0, channel_multiplier=1)
        io_rf = cpool.tile([128, 128], f32)
        io_cf = cpool.tile([128, 1], f32)
        nc.vector.tensor_copy(io_rf[:], io_r[:])
        nc.vector.tensor_copy(io_cf[:], io_c[:])
        ident_h = cpool.tile([128, 128], bf16)
        nc.vector.tensor_scalar(out=ident_h[:], in0=io_rf[:], scalar1=io_cf[:],
                                scalar2=None, op0=Alu.is_equal)
        ident96 = cpool.tile([V, V], f32)
        nc.vector.tensor_scalar(out=ident96[:], in0=io_rf[0:V, 0:V],
                                scalar1=io_cf[0:V, :], scalar2=None,
                                op0=Alu.is_equal)
        ident_r = cpool.tile([128, 128], f32r)
        nc.vector.tensor_scalar(out=ident_r[:], in0=io_rf[:], scalar1=io_cf[:],
                                scalar2=None, op0=Alu.is_equal)

        # ---- weights ----
        w_sb = cpool.tile([128, DI], f32)
        nc.sync.dma_start(out=w_sb[:], in_=w_d[:])
        pos_sb = cpool.tile([V, DO], f32)
        nc.sync.dma_start(out=pos_sb[:], in_=pos_d[:])
        pos_h = cpool.tile([V, DO], f16)
        nc.vector.tensor_copy(pos_h[:], pos_sb[:])
        ones_b = cpool.tile([V, 1], bf16)
        nc.vector.memset(ones_b[:], 1.0)

        # MT chunks [128d, 96v] = (pos_embed @ W)^T in fp16
        with tc.tile_pool(name="pre_psum", bufs=2, space="PSUM") as ppool:
            ps_pt = ppool.tile([128, V], f32, tag="pp")
            nc.tensor.transpose(ps_pt[:], pos_sb[:], ident96[:])
            posT = cpool.tile([128, V], f32)
            nc.vector.tensor_copy(posT[:], ps_pt[:])
            mt_r = []
            for k in range(4):
                ps_mt = ppool.tile([128, V], f32, tag="pp")
                nc.tensor.matmul(ps_mt[:], w_sb[:, k * 128:(k + 1) * 128],
                                 posT[:], start=True, stop=True)
                t = cpool.tile([128, V], f32r, tag=f"mt{k}")
                nc.scalar.copy(t[:], ps_mt[:])
                mt_r.append(t)

        # one-hot compare source [128, 4*96*9] bf16: value v at col (j*864+v*9+c)
        iorep_i = cpool.tile([128, 4 * V * C], i16)
        nc.gpsimd.iota(iorep_i[:], pattern=[[0, 4], [1, V], [0, C]], base=0,
                       channel_multiplier=0)
        iota_rep = cpool.tile([128, 4 * V * C], bf16)
        nc.vector.tensor_copy(iota_rep[:], iorep_i[:])

        # in-length mask [128, CPP] f32: 1.0 where char (64p+i) < len[p//64]
        iota_li = cpool.tile([128, CPP], i32)
        nc.gpsimd.iota(iota_li[:], pattern=[[1, CPP]], base=0,
                       channel_multiplier=CPP)
        len_sb = cpool.tile([128, BLOC], i32)
        nc.sync.dma_start(out=len_sb[:],
                          in_=bass.AP(len_d[:].tensor, 0, [[0, 128], [1, BLOC]]))
        pb = cpool.tile([128, 1], i32)
        nc.vector.tensor_scalar(out=pb[:], in0=io_c[:], scalar1=64,
                                scalar2=None, op0=Alu.is_ge)
        ld = cpool.tile([128, 1], i32)
        nc.vector.tensor_tensor(out=ld[:], in0=len_sb[:, 1:2],
                                in1=len_sb[:, 0:1], op=Alu.subtract)
        ld2 = cpool.tile([128, 1], i32)
        nc.vector.tensor_scalar(out=ld2[:], in0=ld[:], scalar1=L,
                                scalar2=None, op0=Alu.add)
        pbl = cpool.tile([128, 1], i32)
        nc.vector.tensor_tensor(out=pbl[:], in0=pb[:], in1=ld2[:], op=Alu.mult)
        adj = cpool.tile([128, 1], f32)
        nc.vector.tensor_tensor(out=adj[:], in0=len_sb[:, 0:1], in1=pbl[:],
                                op=Alu.add)
        inlen = cpool.tile([128, CPP], f32)
        nc.vector.tensor_scalar(out=inlen[:], in0=iota_li[:], scalar1=adj[:],
                                scalar2=None, op0=Alu.is_lt)

        # whole-core candidate indices / masks (contiguous 128 x 2304B / 576B)
        idx_all = cpool.tile([128, CPP * C], i32)
        nc.scalar.dma_start(
            out=idx_all[:],
            in_=bass.AP(idx_d[:].tensor, 0, [[CPP * C, 128], [1, CPP * C]]))
        msk_all = cpool.tile([128, CPP * C], u8)
        nc.scalar.dma_start(
            out=msk_all[:],
            in_=bass.AP(msk_d[:].tensor, 0, [[CPP * C, 128], [1, CPP * C]]))

        # ---- pools ----
        xpool = es.enter_context(tc.tile_pool(name="x", bufs=2))
        xtpool = es.enter_context(tc.tile_pool(name="xt", bufs=2))
        spool = es.enter_context(tc.tile_pool(name="soft", bufs=2))
        opool = es.enter_context(tc.tile_pool(name="outp", bufs=2))
        ps_xt = es.enter_context(tc.tile_pool(name="ps_xt", bufs=2, space="PSUM"))
        ps_st = es.enter_context(tc.tile_pool(name="ps_st", bufs=2, space="PSUM"))
        ps_ct = es.enter_context(tc.tile_pool(name="ps_ct", bufs=1, space="PSUM"))
        ps_z = es.enter_context(tc.tile_pool(name="ps_z", bufs=1, space="PSUM"))
        ps_cx = es.enter_context(tc.tile_pool(name="ps_cx", bufs=2, space="PSUM"))

        for st in range(NSUP):
            i0 = st * SW
            # x super-tile [128, 8i x 512d] f32r; partition p covers chars 64p+i
            xs = xpool.tile([128, SW * DI], f32r, tag="xs")
            nc.sync.dma_start(
                out=xs[:],
                in_=bass.AP(x_d[:].tensor, i0 * DI,
                            [[CPP * DI, 128], [1, SW * DI]]))
            # PE transpose x -> xT chunks [128 dk, (8 j, 128 n)] per k
            xt = []
            for k in range(4):
                t = xtpool.tile([128, SW * 128], f32r, tag=f"xt{k}")
                for h in range(2):
                    pxt = ps_xt.tile([128, 512], f32r, tag="pxt")
                    for jj in range(4):
                        j = h * 4 + jj
                        nc.tensor.transpose(
                            pxt[:, jj * 128:(jj + 1) * 128],
                            xs[:, j * DI + k * 128:j * DI + (k + 1) * 128],
                            ident_r[:])
                    nc.scalar.copy(t[:, h * 512:(h + 1) * 512], pxt[:])
                xt.append(t)
            # scores s^T [96v, (8j x 128n)]; two psum halves; exp -> et
            et = spool.tile([V, SW * 128], bf16, tag="et")
            for h in range(2):
                pst = ps_st.tile([V, 512], f32, tag="pst")
                for k in range(4):
                    nc.tensor.matmul(pst[:], mt_r[k][:],
                                     xt[k][:, h * 512:(h + 1) * 512],
                                     start=(k == 0), stop=(k == 3))
                nc.scalar.activation(out=et[:, h * 512:(h + 1) * 512],
                                     in_=pst[:], func=Act.Exp,
                                     bias=0.0, scale=1.0)

            # ---- candidate indices -> masked sentinel bf16 ----
            idx_sl = idx_all[:, i0 * C:(i0 + SW) * C]
            msk_sl = msk_all[:, i0 * C:(i0 + SW) * C]
            sent = spool.tile([128, SW * C], i32, tag="sent")
            nc.gpsimd.tensor_scalar(out=sent[:], in0=msk_sl, scalar1=-1000,
                                    scalar2=1000, op0=Alu.mult, op1=Alu.add)
            idxm = spool.tile([128, SW * C], bf16, tag="idxm")
            nc.vector.tensor_tensor(out=idxm[:], in0=idx_sl, in1=sent[:],
                                    op=Alu.add)

            # ---- one-hot eq [128, (j, v, c)] + 2-level tree (Vector) ----
            with nc.allow_low_precision("cnt<=9 exact in bf16"):
                eq = spool.tile([128, SW * V * C], bf16, tag="eq")
                eqv = eq[:].rearrange("p (j v c) -> p j v c", v=V, c=C)
                idx4 = bass.AP(idxm[:].tensor, idxm[:].offset,
                               [idxm[:].ap[0], [C, SW], [0, V], [1, C]])
                iota4 = bass.AP(iota_rep[:].tensor, iota_rep[:].offset,
                                [iota_rep[:].ap[0], [0, SW], [C, V], [1, C]])
                nc.vector.tensor_tensor(out=eqv, in0=iota4, in1=idx4,
                                        op=Alu.is_equal)
                s1 = spool.tile([128, SW * V * 4], bf16, tag="tr_s1")
                s1v = s1[:].rearrange("p (j v c) -> p j v c", v=V, c=4)
                nc.vector.tensor_tensor(out=s1v, in0=eqv[:, :, :, 0:4],
                                        in1=eqv[:, :, :, 4:8], op=Alu.add)
                s2 = spool.tile([128, SW * V * 2], bf16, tag="tr_s2")
                s2v = s2[:].rearrange("p (j v c) -> p j v c", v=V, c=2)
                nc.vector.tensor_tensor(out=s2v, in0=s1v[:, :, :, 0:2],
                                        in1=s1v[:, :, :, 2:4], op=Alu.add)

            # ---- c-tail + transpose fused via accumulating matmuls ----
            cntT = spool.tile([V, SW * 128], bf16, tag="cntT")
            pp_s2 = s2[:].ap[0]
            pp_eq = eq[:].ap[0]
            for h in range(2):
                pct = ps_ct.tile([V, 512], f32, tag="pct")
                for jj in range(4):
                    j = h * 4 + jj
                    srcs = [
                        bass.AP(s2[:].tensor, s2[:].offset + j * V * 2 + 0,
                                [pp_s2, [2, V]]),
                        bass.AP(s2[:].tensor, s2[:].offset + j * V * 2 + 1,
                                [pp_s2, [2, V]]),
                        bass.AP(eq[:].tensor, eq[:].offset + j * V * C + 8,
                                [pp_eq, [C, V]]),
                    ]
                    for t, srcap in enumerate(srcs):
                        nc.tensor.matmul(pct[:, jj * 128:(jj + 1) * 128],
                                         srcap, ident_h[:],
                                         start=(t == 0), stop=(t == 2))
                nc.scalar.copy(cntT[:, h * 512:(h + 1) * 512], pct[:])

            # w^T = cnt^T * e^T (bf16, 2x)
            with nc.allow_low_precision("w bf16"):
                wT = spool.tile([V, SW * 128], bf16, tag="wT")
                nc.vector.tensor_tensor(out=wT[:], in0=cntT[:], in1=et[:],
                                        op=Alu.mult)

            # Z per char via ones-column matmuls -> ps_z [128, 8]
            pz = ps_z.tile([128, SW], f32, tag="pz")
            for j in range(SW):
                nc.tensor.matmul(pz[:, j:j + 1], wT[:, j * 128:(j + 1) * 128],
                                 ones_b[:], start=True, stop=True)
            z = spool.tile([128, SW], f32, tag="z")
            nc.vector.tensor_copy(z[:], pz[:])
            zg = spool.tile([128, SW], f32, tag="zg")
            nc.vector.tensor_scalar(out=zg[:], in0=z[:], scalar1=1e-30,
                                    scalar2=None, op0=Alu.max)
            rz = spool.tile([128, SW], f32, tag="rz")
            nc.vector.reciprocal(rz[:], zg[:])
            rzf = spool.tile([128, SW], f32, tag="rzf")
            nc.vector.tensor_tensor(out=rzf[:], in0=rz[:],
                                    in1=inlen[:, i0:i0 + SW], op=Alu.mult)

            # ctx matmuls + normalized drains (two psum halves)
            outsb = opool.tile([128, SW * 128], f32, tag="outsb")
            for h in range(2):
                pcx = ps_cx.tile([128, 512], f32, tag="pcx")
                for jj in range(4):
                    j = h * 4 + jj
                    nc.tensor.matmul(pcx[:, jj * 128:(jj + 1) * 128],
                                     wT[:, j * 128:(j + 1) * 128],
                                     pos_h[:], start=True, stop=True)
                for jj in range(4):
                    j = h * 4 + jj
                    nc.scalar.activation(out=outsb[:, j * 128:(j + 1) * 128],
                                         in_=pcx[:, jj * 128:(jj + 1) * 128],
                                         func=Act.Copy, bias=0.0,
                                         scale=rzf[:, j:j + 1])
            nc.sync.dma_start(
                out=bass.AP(out_d[:].tensor, i0 * DO,
                            [[CPP * DO, 128], [1, SW * DO]]),
                in_=outsb[:])

    split_excess_waits(nc)
    return nc


_NC_CACHE = None


def kernel(**inputs):
    global _NC_CACHE
    from concourse.bass_utils import run_bass_kernel_spmd

    x = np.ascontiguousarray(inputs["input_context"], dtype=np.float32)
    W = np.ascontiguousarray(inputs["W"], dtype=np.float32)
    pos = np.ascontiguousarray(inputs["pos_embed"], dtype=np.float32)
    idx = np.ascontiguousarray(inputs["cand_idx"], dtype=np.int32)
    msk = np.ascontiguousarray(inputs["cand_mask"]).astype(np.uint8)
    slen = np.ascontiguousarray(inputs["word_seq_len"], dtype=np.int32)

    if _NC_CACHE is None:
        _NC_CACHE = build_kernel()
    nc = _NC_CACHE

    in_maps = []
    for c in range(NCORES):
        b0 = c * BLOC
        in_maps.append({
            "x": x[b0:b0 + BLOC].reshape(NLOC, DI),
            "cand_idx": idx[b0:b0 + BLOC].reshape(NLOC, C),
            "cand_mask": msk[b0:b0 + BLOC].reshape(NLOC, C),
            "W": W,
            "pos_embed": pos,
            "word_seq_len": slen[b0:b0 + BLOC].reshape(1, BLOC),
        })
    res = run_bass_kernel_spmd(nc, in_maps, core_ids=list(range(NCORES)))
    out = np.empty((B, L, DO), np.float32)
    for c in range(NCORES):
        out[c * BLOC:(c + 1) * BLOC] = res.results[c]["out"].reshape(BLOC, L, DO)
    return out
